# revision 9
# baseline (speedup 1.0000x reference)
"""Trainium2 Bass kernel for nn_EnhancedRPTModel (MoE + memory attention + reasoning).

Self-contained: kernel(**inputs) -> np.ndarray.

Sharding: 8-way data-parallel over tokens (512 tokens/core). Activations are
kept feature-major [feat, tok] in SBUF ([128 part, chunks, 512 tok]) so matmuls
chain on the PE without activation transposes (PE contracts over the partition
dim). Dense MoE v1: every core computes all 8 experts for its tokens with
combine weights built on device (bf16 expert weights). Attention K/V are
AllGathered (bf16) within the 4-core group sharing a batch; heads stream with
the O-projection accumulated across heads (linearity). Matmul operands are
bf16; accumulation, softmax, layernorm stats and the residual stream are f32.
"""
import numpy as np
import ml_dtypes

import concourse.bass as bass
import concourse.bacc as bacc
import concourse.mybir as mybir
import concourse.tile as tile
from concourse.bass_utils import run_bass_kernel_spmd
from concourse.masks import make_identity

dt = mybir.dt
F32 = dt.float32
BF16 = dt.bfloat16

B, S, H = 2, 2048, 2048
E, K_TOP, HID = 8, 2, 4096
NH, HD = 8, 256
MS, MD = 256, 512
RSTEPS, RD = 3, 512
HG = H // 4
SCALE = 16.0

NCORES = 8
T = (B * S) // NCORES          # 512 tokens per core
TT = T // 128                  # 4 token tiles
F = H // 128                   # 16 feature chunks
FH = HID // 128                # 32 hidden chunks

_NC_CACHE = {}


def ts(i, size):
    return slice(i * size, (i + 1) * size)


def _rw(ap):
    return ap.rearrange("(f p) c -> p f c", p=128)


def _rb(ap):
    return ap.rearrange("(f p) -> p f", p=128)


def build_nc():
    nc = bacc.Bacc("TRN2", target_bir_lowering=False, debug=False, num_devices=NCORES)

    def inp(name, shape, dtype=F32):
        return nc.dram_tensor(name, shape, dtype, kind="ExternalInput").ap()

    xT = inp("xT", [H, T])
    mask = inp("mask", [1, S])
    gate_w = inp("gate_w", [H, E])
    gate_b = inp("gate_b", [1, E])
    moe_w1 = inp("moe_w1", [E, H, HID], BF16)
    moe_b1 = inp("moe_b1", [E, HID])
    moe_w2 = inp("moe_w2", [E, HID, H], BF16)
    moe_b2 = inp("moe_b2", [E, H])
    q_w = inp("q_w", [H, H], BF16); q_b = inp("q_b", [H])
    k_w = inp("k_w", [H, H], BF16); k_b = inp("k_b", [H])
    v_w = inp("v_w", [H, H], BF16); v_b = inp("v_b", [1, H])
    o_w = inp("o_w", [H, H], BF16); o_b = inp("o_b", [H])
    mem_values = inp("mem_values", [MS, MD])
    mem_proj_w = inp("mem_proj_w", [MD, H], BF16); mem_proj_b = inp("mem_proj_b", [H])
    mem_attn_w = inp("mem_attn_w", [H, MS], BF16); mem_attn_b = inp("mem_attn_b", [1, MS])
    rs_w1 = inp("rs_w1", [RSTEPS, H, RD], BF16); rs_b1 = inp("rs_b1", [RSTEPS, RD])
    rs_w2 = inp("rs_w2", [RSTEPS, RD, H], BF16); rs_b2 = inp("rs_b2", [RSTEPS, H])
    ln_g = inp("ln_g", [RSTEPS, H]); ln_b = inp("ln_b", [RSTEPS, H])
    hg_w1 = inp("hg_w1", [RSTEPS, H, HG], BF16); hg_b1 = inp("hg_b1", [RSTEPS, HG])
    hg_w2 = inp("hg_w2", [RSTEPS, HG, 1], BF16); hg_b2 = inp("hg_b2", [RSTEPS, 1])
    integ_w = inp("integ_w", [RSTEPS * H, H], BF16); integ_b = inp("integ_b", [H])

    out = nc.dram_tensor("out", [H, T], F32, kind="ExternalOutput").ap()

    Exp = mybir.ActivationFunctionType.Exp
    Relu = mybir.ActivationFunctionType.Relu
    Ident = mybir.ActivationFunctionType.Identity
    Sqrt = mybir.ActivationFunctionType.Sqrt
    Square = mybir.ActivationFunctionType.Square
    Sigmoid = mybir.ActivationFunctionType.Sigmoid
    mult = mybir.AluOpType.mult
    add = mybir.AluOpType.add
    is_ge = mybir.AluOpType.is_ge
    is_equal = mybir.AluOpType.is_equal
    AXX = mybir.AxisListType.X
    MAX = mybir.AluOpType.max

    with tile.TileContext(nc) as tc:
      with (
        tc.tile_pool(name="const", bufs=1) as constp,
        tc.tile_pool(name="hpool", bufs=1) as hpool,
        tc.tile_pool(name="dram", bufs=1, space="DRAM") as dramp,
      ):
        ident = constp.tile([128, 128], F32)
        make_identity(nc, ident)
        ones1 = constp.tile([1, 128], F32)
        nc.vector.memset(ones1[:], 1.0)
        ones128b = constp.tile([128, 1], BF16)
        nc.vector.memset(ones128b[:], 1.0)
        ones128f = constp.tile([128, 1], F32)
        nc.vector.memset(ones128f[:], 1.0)

        h = hpool.tile([128, F, T], F32)   # residual stream; doubles as MoE accumulator

        # =============== gate + dense MoE + residual ===============
        with (
            tc.tile_pool(name="px", bufs=1) as px,
            tc.tile_pool(name="pw", bufs=2) as pw,
            tc.tile_pool(name="pev", bufs=2) as pev,
            tc.tile_pool(name="pps", bufs=4, space="PSUM") as pps,
            tc.tile_pool(name="ppsg", bufs=1, space="PSUM") as ppsg,
        ):
            xTt = px.tile([128, F, T], F32)
            nc.sync.dma_start(out=xTt[:], in_=xT.rearrange("(f p) t -> p f t", p=128))
            xTbf = px.tile([128, F, T], BF16)
            for f in range(F):
                nc.vector.tensor_copy(xTbf[:, f, :], xTt[:, f, :])

            gate_w_sb = px.tile([128, F, E], F32)
            nc.sync.dma_start(out=gate_w_sb[:], in_=_rw(gate_w))
            gate_b_sb = px.tile([1, E], F32)
            nc.sync.dma_start(out=gate_b_sb[:], in_=gate_b[:])
            combT = px.tile([E, T], F32)

            for t in range(TT):
                gps = ppsg.tile([128, E], F32, tag="gps")
                for k in range(F):
                    nc.tensor.matmul(gps[:], xTt[:, k, ts(t, 128)], gate_w_sb[:, k, :],
                                     start=(k == 0), stop=False)
                nc.tensor.matmul(gps[:], ones1[:], gate_b_sb[:], start=False, stop=True)
                mx = pev.tile([128, 1], F32, tag="g1")
                nc.vector.tensor_reduce(out=mx[:], in_=gps[:], op=MAX, axis=AXX,
                                        negate=True)
                probs = pev.tile([128, E], F32, tag="gp")
                ssum = pev.tile([128, 1], F32, tag="g2")
                nc.scalar.activation(probs[:], gps[:], Exp, bias=mx[:, :1],
                                     accum_out=ssum[:])
                rsum = pev.tile([128, 1], F32, tag="g3")
                nc.vector.reciprocal(rsum[:], ssum[:])
                nc.vector.tensor_scalar(probs[:], probs[:], rsum[:, :1], None, op0=mult)
                m1 = pev.tile([128, 1], F32, tag="g4")
                nc.vector.tensor_reduce(out=m1[:], in_=probs[:], op=MAX, axis=AXX)
                ismax = pev.tile([128, E], F32, tag="g5")
                nc.vector.tensor_scalar(ismax[:], probs[:], m1[:, :1], None, op0=is_equal)
                pm = pev.tile([128, E], F32, tag="g6")
                nc.vector.tensor_sub(pm[:], probs[:], ismax[:])
                m2 = pev.tile([128, 1], F32, tag="g7")
                nc.vector.tensor_reduce(out=m2[:], in_=pm[:], op=MAX, axis=AXX)
                sel = pev.tile([128, E], F32, tag="g8")
                nc.vector.tensor_scalar(sel[:], probs[:], m2[:, :1], None, op0=is_ge)
                e12 = pev.tile([128, 2], F32, tag="g9")
                nc.scalar.activation(e12[:, 0:1], m1[:], Exp)
                nc.scalar.activation(e12[:, 1:2], m2[:], Exp)
                esum = pev.tile([128, 1], F32, tag="g10")
                nc.vector.tensor_reduce(out=esum[:], in_=e12[:], op=add, axis=AXX)
                erec = pev.tile([128, 1], F32, tag="g11")
                nc.vector.reciprocal(erec[:], esum[:])
                expp = pev.tile([128, E], F32, tag="g12")
                nc.scalar.activation(expp[:], probs[:], Exp)
                comb = pev.tile([128, E], F32, tag="g13")
                nc.vector.tensor_mul(comb[:], sel[:], expp[:])
                nc.vector.tensor_scalar(comb[:], comb[:], erec[:, :1], 0.5,
                                        op0=mult, op1=mult)
                ctp = ppsg.tile([E, 128], F32, tag="ctp")
                nc.tensor.transpose(out=ctp[:], in_=comb[:], identity=ident[:])
                nc.scalar.copy(combT[:, ts(t, 128)], ctp[:])

            h1 = px.tile([128, FH, T], BF16)
            for e in range(E):
                wrow = pev.tile([1, T], F32, tag="wrow")
                nc.sync.dma_start(out=wrow[:], in_=combT[e:e + 1, :])
                wbp = ppsg.tile([128, T], F32, tag="wbp")
                nc.tensor.matmul(wbp[:], ones1[:], wrow[:], start=True, stop=True)
                wb = pev.tile([128, T], F32, tag="wb")
                nc.scalar.copy(wb[:], wbp[:])
                b1_sb = pev.tile([128, FH], F32, tag="b1")
                nc.sync.dma_start(out=b1_sb[:], in_=_rb(moe_b1[e]))
                b2_sb = pev.tile([128, F], F32, tag="b2")
                nc.sync.dma_start(out=b2_sb[:], in_=_rb(moe_b2[e]))

                for s in range(8):
                    w1s = pw.tile([128, F, 512], BF16, tag="wmoe")
                    nc.sync.dma_start(out=w1s[:], in_=_rw(moe_w1[e])[:, :, ts(s, 512)])
                    for m in range(4):
                        mi = s * 4 + m
                        ps = pps.tile([128, T], F32, tag="mm")
                        for k in range(F):
                            nc.tensor.matmul(ps[:], w1s[:, k, ts(m, 128)], xTbf[:, k, :],
                                             start=(k == 0), stop=(k == F - 1))
                        nc.scalar.activation(h1[:, mi, :], ps[:], Relu,
                                             bias=b1_sb[:, mi:mi + 1])
                for s in range(8):
                    w2s = pw.tile([128, FH, 256], BF16, tag="wmoe")
                    nc.sync.dma_start(out=w2s[:], in_=_rw(moe_w2[e])[:, :, ts(s, 256)])
                    for m in range(2):
                        mi = s * 2 + m
                        ps = pps.tile([128, T], F32, tag="mm")
                        for k in range(FH):
                            nc.tensor.matmul(ps[:], w2s[:, k, ts(m, 128)], h1[:, k, :],
                                             start=(k == 0), stop=(k == FH - 1))
                        eo = pev.tile([128, T], F32, tag="eo")
                        nc.scalar.activation(eo[:], ps[:], Ident, bias=b2_sb[:, mi:mi + 1])
                        if e == 0:
                            nc.vector.tensor_mul(h[:, mi, :], eo[:], wb[:])
                        else:
                            nc.vector.tensor_mul(eo[:], eo[:], wb[:])
                            nc.vector.tensor_add(h[:, mi, :], h[:, mi, :], eo[:])
            for f in range(F):
                nc.vector.tensor_add(h[:, f, :], h[:, f, :], xTt[:, f, :])

        # =============== attention + memory + o-proj ===============
        with (
            tc.tile_pool(name="pattn", bufs=1) as pattn,
            tc.tile_pool(name="pw2", bufs=2) as pw2,
            tc.tile_pool(name="pps2", bufs=2, space="PSUM") as pps2,
        ):
            h_bf = pattn.tile([128, F, T], BF16)
            for f in range(F):
                nc.vector.tensor_copy(h_bf[:, f, :], h[:, f, :])
            o_acc = pattn.tile([128, F, T], F32)
            mneg_bc = pattn.tile([128, S], F32)

            kv_in = dramp.tile([2, 128, F * T], BF16)
            kv_out = dramp.tile([4, 2, 128, F * T], BF16)

            with (
                tc.tile_pool(name="pkv", bufs=1) as pkv,
                tc.tile_pool(name="pev0", bufs=2) as pev0,
            ):
                k_sb = pkv.tile([128, F, T], BF16)
                v_sb = pkv.tile([128, TT, H], BF16)
                kb_sb = pev0.tile([128, F], F32, tag="kb")
                nc.sync.dma_start(out=kb_sb[:], in_=_rb(k_b))
                for s in range(4):
                    ws = pw2.tile([128, F, 512], BF16, tag="wproj")
                    nc.sync.dma_start(out=ws[:], in_=_rw(k_w)[:, :, ts(s, 512)])
                    for m in range(4):
                        mi = s * 4 + m
                        ps = pps2.tile([128, T], F32, tag="mm")
                        for k in range(F):
                            nc.tensor.matmul(ps[:], ws[:, k, ts(m, 128)], h_bf[:, k, :],
                                             start=(k == 0), stop=(k == F - 1))
                        nc.scalar.activation(k_sb[:, mi, :], ps[:], Ident,
                                             bias=kb_sb[:, mi:mi + 1])
                vb_sb = pev0.tile([1, H], F32, tag="vb")
                nc.sync.dma_start(out=vb_sb[:], in_=v_b[:])
                for s in range(4):
                    ws = pw2.tile([128, F, 512], BF16, tag="wproj")
                    nc.sync.dma_start(out=ws[:], in_=_rw(v_w)[:, :, ts(s, 512)])
                    for t in range(TT):
                        ps = pps2.tile([128, 512], F32, tag="mm")
                        for k in range(F):
                            nc.tensor.matmul(ps[:], h_bf[:, k, ts(t, 128)], ws[:, k, :],
                                             start=(k == 0), stop=False)
                        nc.tensor.matmul(ps[:], ones1[:], vb_sb[:, ts(s, 512)],
                                         start=False, stop=True)
                        nc.scalar.copy(v_sb[:, t, ts(s, 512)], ps[:])
                nc.sync.dma_start(out=kv_in[0], in_=k_sb[:].rearrange("p f t -> p (f t)"))
                nc.sync.dma_start(out=kv_in[1], in_=v_sb[:].rearrange("p a b -> p (a b)"))
                nc.gpsimd.collective_compute(
                    "AllGather", mybir.AluOpType.bypass,
                    replica_groups=[[0, 1, 2, 3], [4, 5, 6, 7]],
                    ins=[kv_in.opt()], outs=[kv_out.opt()],
                )
                mask_sb = pev0.tile([1, S], F32, tag="msk")
                nc.sync.dma_start(out=mask_sb[:], in_=mask[:])
                nc.vector.tensor_scalar_mul(mask_sb[:], mask_sb[:], -1e9)
                for s in range(4):
                    bps = pps2.tile([128, 512], F32, tag="mm")
                    nc.tensor.matmul(bps[:], ones1[:], mask_sb[:, ts(s, 512)],
                                     start=True, stop=True)
                    nc.scalar.copy(mneg_bc[:, ts(s, 512)], bps[:])

            with tc.tile_pool(name="phd", bufs=1) as phd:
                qb_sb = phd.tile([128, F], F32)
                nc.sync.dma_start(out=qb_sb[:], in_=_rb(q_b))
                for hh in range(NH):
                    qws = pw2.tile([128, F, 256], BF16, tag="wproj")
                    nc.sync.dma_start(out=qws[:], in_=_rw(q_w)[:, :, ts(hh, 256)])
                    q_head = phd.tile([128, 2, T], BF16, tag="qh", bufs=2)
                    for m in range(2):
                        ps = pps2.tile([128, T], F32, tag="mm")
                        for k in range(F):
                            nc.tensor.matmul(ps[:], qws[:, k, ts(m, 128)], h_bf[:, k, :],
                                             start=(k == 0), stop=(k == F - 1))
                        nc.scalar.activation(q_head[:, m, :], ps[:], Ident,
                                             bias=qb_sb[:, hh * 2 + m:hh * 2 + m + 1])
                    k_head = phd.tile([128, 2, 4, 512], BF16, tag="kh", bufs=2)
                    v_head = phd.tile([128, 16, 256], BF16, tag="vh", bufs=2)
                    for r in range(4):
                        nc.sync.dma_start(
                            out=k_head[:, :, r, :],
                            in_=kv_out[r, 0].rearrange("p (f t) -> p f t", f=F)[:, 2 * hh:2 * hh + 2, :])
                        nc.sync.dma_start(
                            out=v_head[:, ts(r, 4), :],
                            in_=kv_out[r, 1].rearrange("p (a b) -> p a b", a=TT)[:, :, ts(hh, 256)])
                    ows = pw2.tile([128, 2, H], BF16, tag="wo")
                    nc.sync.dma_start(out=ows[:], in_=_rw(o_w)[:, 2 * hh:2 * hh + 2, :])
                    attn_h = phd.tile([128, 2, T], BF16, tag="ah", bufs=2)
                    for t in range(TT):
                        pslist = []
                        for r in range(4):
                            pss = pps2.tile([128, 512], F32, tag="sc", bufs=4)
                            for c in range(2):
                                nc.tensor.matmul(pss[:], q_head[:, c, ts(t, 128)],
                                                 k_head[:, c, r, :],
                                                 start=(c == 0), stop=(c == 1))
                            nc.vector.tensor_add(pss[:], pss[:], mneg_bc[:, ts(r, 512)])
                            pslist.append(pss)
                        mx4 = phd.tile([128, 4], F32, tag="mx4", bufs=2)
                        for r in range(4):
                            nc.vector.tensor_reduce(out=mx4[:, r:r + 1], in_=pslist[r][:],
                                                    op=MAX, axis=AXX)
                        negmax = phd.tile([128, 1], F32, tag="negmax", bufs=2)
                        nc.vector.tensor_reduce(out=negmax[:], in_=mx4[:], op=MAX,
                                                axis=AXX, negate=True)
                        nc.vector.tensor_scalar_mul(negmax[:], negmax[:], 1.0 / SCALE)
                        probs = phd.tile([128, 4, 512], F32, tag="probs", bufs=2)
                        sums4 = phd.tile([128, 4], F32, tag="sums4", bufs=2)
                        for r in range(4):
                            nc.scalar.activation(probs[:, r, :], pslist[r][:], Exp,
                                                 bias=negmax[:, :1], scale=1.0 / SCALE,
                                                 accum_out=sums4[:, r:r + 1])
                        rs_ = phd.tile([128, 1], F32, tag="rs", bufs=2)
                        nc.vector.tensor_reduce(out=rs_[:], in_=sums4[:], op=add, axis=AXX)
                        nc.vector.reciprocal(rs_[:], rs_[:])
                        nc.vector.tensor_scalar(
                            probs[:].rearrange("p a b -> p (a b)"),
                            probs[:].rearrange("p a b -> p (a b)"),
                            rs_[:, :1], None, op0=mult)
                        probsT = phd.tile([128, 16, 128], BF16, tag="probsT", bufs=2)
                        for r in range(4):
                            for j in range(4):
                                tps = pps2.tile([128, 128], F32, tag="tp", bufs=2)
                                nc.tensor.transpose(out=tps[:], in_=probs[:, r, ts(j, 128)],
                                                    identity=ident[:])
                                nc.scalar.copy(probsT[:, r * 4 + j, :], tps[:])
                        for m in range(2):
                            pav = pps2.tile([128, 128], F32, tag="tp", bufs=2)
                            for kc in range(16):
                                nc.tensor.matmul(pav[:], v_head[:, kc, ts(m, 128)],
                                                 probsT[:, kc, :],
                                                 start=(kc == 0), stop=(kc == 15))
                            nc.scalar.copy(attn_h[:, m, ts(t, 128)], pav[:])
                    # o-proj contribution of this head
                    for mi in range(F):
                        ps = pps2.tile([128, T], F32, tag="mm")
                        for kc in range(2):
                            nc.tensor.matmul(ps[:], ows[:, kc, ts(mi, 128)],
                                             attn_h[:, kc, :],
                                             start=(kc == 0), stop=(kc == 1))
                        if hh == 0:
                            nc.scalar.copy(o_acc[:, mi, :], ps[:])
                        else:
                            nc.vector.tensor_add(o_acc[:, mi, :], o_acc[:, mi, :], ps[:])

            # ---- memory attention (0.3*mem contribution via o_w linearity) ----
            with tc.tile_pool(name="pmem", bufs=1) as pmem:
                maw_sb = pmem.tile([128, F, MS], BF16)
                nc.sync.dma_start(out=maw_sb[:], in_=_rw(mem_attn_w))
                mab_sb = pmem.tile([1, MS], F32)
                nc.sync.dma_start(out=mab_sb[:], in_=mem_attn_b[:])
                mab_bc = pmem.tile([128, MS], F32)
                bps = pps2.tile([128, 512], F32, tag="mm")
                nc.tensor.matmul(bps[:, :MS], ones1[:], mab_sb[:], start=True, stop=True)
                nc.scalar.copy(mab_bc[:], bps[:, :MS])
                memv_sb = pmem.tile([128, 2, MD], F32)
                nc.sync.dma_start(out=memv_sb[:], in_=_rw(mem_values))
                mavT = pmem.tile([128, 4, T], BF16)
                for t in range(TT):
                    psml = pps2.tile([128, MS], F32, tag="mm")
                    for k in range(F):
                        nc.tensor.matmul(psml[:], h_bf[:, k, ts(t, 128)], maw_sb[:, k, :],
                                         start=(k == 0), stop=(k == F - 1))
                    nc.vector.tensor_add(psml[:], psml[:], mab_bc[:])
                    negmax = pmem.tile([128, 1], F32, tag="mn", bufs=2)
                    nc.vector.tensor_reduce(out=negmax[:], in_=psml[:], op=MAX, axis=AXX,
                                            negate=True)
                    memp = pmem.tile([128, MS], F32, tag="memp", bufs=2)
                    msum = pmem.tile([128, 1], F32, tag="msum", bufs=2)
                    nc.scalar.activation(memp[:], psml[:], Exp, bias=negmax[:, :1],
                                         accum_out=msum[:])
                    nc.vector.reciprocal(msum[:], msum[:])
                    nc.vector.tensor_scalar(memp[:], memp[:], msum[:, :1], None, op0=mult)
                    mempT = pmem.tile([128, 2, 128], F32, tag="mempT", bufs=2)
                    for j in range(2):
                        tps = pps2.tile([128, 128], F32, tag="tp", bufs=2)
                        nc.tensor.transpose(out=tps[:], in_=memp[:, ts(j, 128)],
                                            identity=ident[:])
                        nc.scalar.copy(mempT[:, j, :], tps[:])
                    for m in range(4):
                        pmv = pps2.tile([128, 128], F32, tag="tp", bufs=2)
                        for kc in range(2):
                            nc.tensor.matmul(pmv[:], memv_sb[:, kc, ts(m, 128)],
                                             mempT[:, kc, :],
                                             start=(kc == 0), stop=(kc == 1))
                        nc.scalar.copy(mavT[:, m, ts(t, 128)], pmv[:])
                mem_oT = pmem.tile([128, F, T], BF16)
                mpb_sb = pmem.tile([128, F], F32)
                nc.sync.dma_start(out=mpb_sb[:], in_=_rb(mem_proj_b))
                nc.vector.tensor_scalar_mul(mpb_sb[:], mpb_sb[:], 0.3)
                for s in range(4):
                    mpw_s = pw2.tile([128, 4, 512], BF16, tag="wo")
                    nc.sync.dma_start(out=mpw_s[:], in_=_rw(mem_proj_w)[:, :, ts(s, 512)])
                    for m in range(4):
                        mi = s * 4 + m
                        ps = pps2.tile([128, T], F32, tag="mm")
                        for kc in range(4):
                            nc.tensor.matmul(ps[:], mpw_s[:, kc, ts(m, 128)], mavT[:, kc, :],
                                             start=(kc == 0), stop=(kc == 3))
                        nc.scalar.activation(mem_oT[:, mi, :], ps[:], Ident,
                                             bias=mpb_sb[:, mi:mi + 1], scale=0.3)
                for s in range(4):
                    wos = pw2.tile([128, F, 512], BF16, tag="wproj")
                    nc.sync.dma_start(out=wos[:], in_=_rw(o_w)[:, :, ts(s, 512)])
                    for m in range(4):
                        mi = s * 4 + m
                        ps = pps2.tile([128, T], F32, tag="mm")
                        for k in range(F):
                            nc.tensor.matmul(ps[:], wos[:, k, ts(m, 128)], mem_oT[:, k, :],
                                             start=(k == 0), stop=(k == F - 1))
                        nc.vector.tensor_add(o_acc[:, mi, :], o_acc[:, mi, :], ps[:])
                ob_sb = pmem.tile([128, F], F32)
                nc.sync.dma_start(out=ob_sb[:], in_=_rb(o_b))
                for mi in range(F):
                    tmp = pmem.tile([128, T], F32, tag="tmp", bufs=2)
                    nc.scalar.activation(tmp[:], o_acc[:, mi, :], Ident,
                                         bias=ob_sb[:, mi:mi + 1])
                    nc.vector.tensor_add(h[:, mi, :], h[:, mi, :], tmp[:])

        # =============== hierarchical reasoning + integration ===============
        with (
            tc.tile_pool(name="prs", bufs=1) as prs,
            tc.tile_pool(name="pw3", bufs=2) as pw3,
            tc.tile_pool(name="pev3", bufs=1) as pev3,
            tc.tile_pool(name="pps3", bufs=4, space="PSUM") as pps3,
            tc.tile_pool(name="ppsc", bufs=1, space="PSUM") as ppsc,
        ):
            cur = prs.tile([128, F, T], BF16)
            for f in range(F):
                nc.vector.tensor_copy(cur[:, f, :], h[:, f, :])
            integ_acc = prs.tile([128, F, T], F32)
            so = prs.tile([128, F, T], BF16)

            for i in range(RSTEPS):
                rb1_sb = pev3.tile([128, 4], F32, tag="rb1")
                nc.sync.dma_start(out=rb1_sb[:], in_=_rb(rs_b1[i]))
                s1 = pev3.tile([128, 4, T], BF16, tag="s1")
                for s in range(2):
                    rs1_sb = pw3.tile([128, F, 256], BF16, tag="w1")
                    nc.sync.dma_start(out=rs1_sb[:], in_=_rw(rs_w1[i])[:, :, ts(s, 256)])
                    for m in range(2):
                        mi = s * 2 + m
                        ps = pps3.tile([128, T], F32, tag="mm")
                        for k in range(F):
                            nc.tensor.matmul(ps[:], rs1_sb[:, k, ts(m, 128)], cur[:, k, :],
                                             start=(k == 0), stop=(k == F - 1))
                        nc.scalar.activation(s1[:, mi, :], ps[:], Relu,
                                             bias=rb1_sb[:, mi:mi + 1])
                rb2_sb = pev3.tile([128, F], F32, tag="rb2")
                nc.sync.dma_start(out=rb2_sb[:], in_=_rb(rs_b2[i]))
                for s in range(4):
                    rs2_sb = pw3.tile([128, 4, 512], BF16, tag="w2")
                    nc.sync.dma_start(out=rs2_sb[:], in_=_rw(rs_w2[i])[:, :, ts(s, 512)])
                    for m in range(4):
                        mi = s * 4 + m
                        ps = pps3.tile([128, T], F32, tag="mm")
                        for k in range(4):
                            nc.tensor.matmul(ps[:], rs2_sb[:, k, ts(m, 128)], s1[:, k, :],
                                             start=(k == 0), stop=(k == 3))
                        nc.scalar.activation(so[:, mi, :], ps[:], Ident,
                                             bias=rb2_sb[:, mi:mi + 1])
                # layernorm stats via ones-matmul column sums
                psum_s = ppsc.tile([1, T], F32, tag="cs1")
                psum_q = ppsc.tile([1, T], F32, tag="cs2")
                for mi in range(F):
                    nc.tensor.matmul(psum_s[:], ones128b[:], so[:, mi, :],
                                     start=(mi == 0), stop=(mi == F - 1))
                sqt = pev3.tile([128, T], F32, tag="sqt", bufs=2)
                for mi in range(F):
                    nc.scalar.activation(sqt[:], so[:, mi, :], Square)
                    nc.tensor.matmul(psum_q[:], ones128f[:], sqt[:],
                                     start=(mi == 0), stop=(mi == F - 1))
                mu = pev3.tile([1, T], F32, tag="mu")
                nc.scalar.mul(mu[:], psum_s[:], 1.0 / H)
                msq = pev3.tile([1, T], F32, tag="msq")
                nc.scalar.mul(msq[:], psum_q[:], 1.0 / H)
                var = pev3.tile([1, T], F32, tag="var")
                nc.vector.tensor_mul(var[:], mu[:], mu[:])
                nc.vector.tensor_sub(var[:], msq[:], var[:])
                nc.vector.tensor_scalar_add(var[:], var[:], 1e-5)
                sd = pev3.tile([1, T], F32, tag="sd")
                nc.scalar.activation(sd[:], var[:], Sqrt)
                rstd = pev3.tile([1, T], F32, tag="rstd")
                nc.vector.reciprocal(rstd[:], sd[:])
                mub = pev3.tile([128, T], BF16, tag="mub")
                rstdb = pev3.tile([128, T], BF16, tag="rstdb")
                for (src, dst) in ((mu, mub), (rstd, rstdb)):
                    bps2 = ppsc.tile([128, T], F32, tag="bc")
                    nc.tensor.matmul(bps2[:], ones1[:], src[:], start=True, stop=True)
                    nc.scalar.copy(dst[:], bps2[:])
                # hier gate
                hgb1_sb = pev3.tile([128, 4], F32, tag="hgb1")
                nc.sync.dma_start(out=hgb1_sb[:], in_=_rb(hg_b1[i]))
                a1 = pev3.tile([128, 4, T], BF16, tag="s1")
                for s in range(2):
                    hg1_sb = pw3.tile([128, F, 256], BF16, tag="w1")
                    nc.sync.dma_start(out=hg1_sb[:], in_=_rw(hg_w1[i])[:, :, ts(s, 256)])
                    for m in range(2):
                        mi = s * 2 + m
                        ps = pps3.tile([128, T], F32, tag="mm")
                        for k in range(F):
                            nc.tensor.matmul(ps[:], hg1_sb[:, k, ts(m, 128)], cur[:, k, :],
                                             start=(k == 0), stop=(k == F - 1))
                        nc.scalar.activation(a1[:, mi, :], ps[:], Relu,
                                             bias=hgb1_sb[:, mi:mi + 1])
                hg2_sb = pev3.tile([128, 4, 1], BF16, tag="hg2")
                nc.sync.dma_start(out=hg2_sb[:], in_=hg_w2[i].rearrange("(k p) o -> p k o", p=128))
                hgb2_sb = pev3.tile([1, 1], F32, tag="hgb2")
                nc.sync.dma_start(out=hgb2_sb[:], in_=hg_b2[i:i + 1])
                psg = ppsc.tile([1, T], F32, tag="cs1")
                for k in range(4):
                    nc.tensor.matmul(psg[:], hg2_sb[:, k, :], a1[:, k, :],
                                     start=(k == 0), stop=(k == 3))
                gsig = pev3.tile([1, T], F32, tag="gsig")
                nc.scalar.activation(gsig[:], psg[:], Sigmoid, bias=hgb2_sb[:, :1])
                gb = pev3.tile([128, T], BF16, tag="gb")
                bps2 = ppsc.tile([128, T], F32, tag="bc")
                nc.tensor.matmul(bps2[:], ones1[:], gsig[:], start=True, stop=True)
                nc.scalar.copy(gb[:], bps2[:])
                # normalize + gate + update cur
                lng_sb = pev3.tile([128, F], F32, tag="lng")
                nc.sync.dma_start(out=lng_sb[:], in_=_rb(ln_g[i]))
                lnb_sb = pev3.tile([128, F], F32, tag="lnb")
                nc.sync.dma_start(out=lnb_sb[:], in_=_rb(ln_b[i]))
                for mi in range(F):
                    t1 = pev3.tile([128, T], BF16, tag="t1", bufs=2)
                    nc.vector.tensor_sub(t1[:], so[:, mi, :], mub[:])
                    nc.vector.tensor_mul(t1[:], t1[:], rstdb[:])
                    nc.vector.tensor_scalar(t1[:], t1[:], lng_sb[:, mi:mi + 1],
                                            lnb_sb[:, mi:mi + 1], op0=mult, op1=add)
                    nc.vector.tensor_mul(t1[:], t1[:], gb[:])
                    nc.vector.tensor_add(cur[:, mi, :], cur[:, mi, :], t1[:])
                # integration block i
                for s in range(8):
                    iw_s = pw3.tile([128, F, 256], BF16, tag="wi")
                    nc.sync.dma_start(out=iw_s[:], in_=_rw(integ_w[ts(i, H)])[:, :, ts(s, 256)])
                    for m in range(2):
                        mi = s * 2 + m
                        ps = pps3.tile([128, T], F32, tag="mm")
                        for k in range(F):
                            nc.tensor.matmul(ps[:], iw_s[:, k, ts(m, 128)], cur[:, k, :],
                                             start=(k == 0), stop=(k == F - 1))
                        if i == 0:
                            nc.scalar.copy(integ_acc[:, mi, :], ps[:])
                        else:
                            nc.vector.tensor_add(integ_acc[:, mi, :], integ_acc[:, mi, :], ps[:])

            ib_sb = pev3.tile([128, F], F32, tag="ib")
            nc.sync.dma_start(out=ib_sb[:], in_=_rb(integ_b))
            outt = prs.tile([128, F, T], F32)
            for mi in range(F):
                tmp = pev3.tile([128, T], F32, tag="tmpo", bufs=2)
                nc.scalar.activation(tmp[:], integ_acc[:, mi, :], Ident,
                                     bias=ib_sb[:, mi:mi + 1])
                nc.vector.tensor_add(outt[:, mi, :], h[:, mi, :], tmp[:])
            nc.sync.dma_start(out=out.rearrange("(f p) t -> p f t", p=128), in_=outt[:])

    nc.compile()
    return nc


def _get_nc():
    if "nc" not in _NC_CACHE:
        _NC_CACHE["nc"] = build_nc()
    return _NC_CACHE["nc"]


def kernel(**inputs):
    nc = _get_nc()
    x = np.asarray(inputs["hidden_states"], np.float32)
    mask = np.asarray(inputs["attention_mask"], np.float32)
    x_flat = x.reshape(B * S, H)
    xT_full = np.ascontiguousarray(x_flat.T)

    def f32(name, shape=None):
        a = np.ascontiguousarray(np.asarray(inputs[name], np.float32))
        return a.reshape(shape) if shape is not None else a

    def bf16(name):
        return np.ascontiguousarray(
            np.asarray(inputs[name], np.float32).astype(ml_dtypes.bfloat16))

    shared = {
        "gate_w": f32("gate_w"), "gate_b": f32("gate_b", (1, E)),
        "moe_w1": bf16("moe_w1"), "moe_b1": f32("moe_b1"),
        "moe_w2": bf16("moe_w2"), "moe_b2": f32("moe_b2"),
        "q_w": bf16("q_w"), "q_b": f32("q_b"),
        "k_w": bf16("k_w"), "k_b": f32("k_b"),
        "v_w": bf16("v_w"), "v_b": f32("v_b", (1, H)),
        "o_w": bf16("o_w"), "o_b": f32("o_b"),
        "mem_values": f32("mem_values"),
        "mem_proj_w": bf16("mem_proj_w"), "mem_proj_b": f32("mem_proj_b"),
        "mem_attn_w": bf16("mem_attn_w"), "mem_attn_b": f32("mem_attn_b", (1, MS)),
        "rs_w1": bf16("rs_w1"), "rs_b1": f32("rs_b1"),
        "rs_w2": bf16("rs_w2"), "rs_b2": f32("rs_b2"),
        "ln_g": f32("ln_g"), "ln_b": f32("ln_b"),
        "hg_w1": bf16("hg_w1"), "hg_b1": f32("hg_b1"),
        "hg_w2": bf16("hg_w2"), "hg_b2": f32("hg_b2"),
        "integ_w": bf16("integ_w"), "integ_b": f32("integ_b"),
    }

    in_maps = []
    for c in range(NCORES):
        b = c // (NCORES // B)
        m = {"xT": np.ascontiguousarray(xT_full[:, c * T:(c + 1) * T]),
             "mask": np.ascontiguousarray(mask[b].reshape(1, S))}
        m.update(shared)
        in_maps.append(m)

    res = run_bass_kernel_spmd(nc, in_maps, list(range(NCORES)))
    outT = np.concatenate([res.results[c]["out"] for c in range(NCORES)], axis=1)
    return np.ascontiguousarray(outT.T).reshape(B, S, H).astype(np.float32)


if __name__ == "__main__":
    _get_nc()
    print("compiled ok")
